# revision 1
# baseline (speedup 1.0000x reference)
import sys

sys.path.insert(0, "/opt/trn_rl_repo")

import numpy as np

NCORES = 8
B, FULL_N, D = 4, 2048, 1024
NH = 16
DK = 64  # head dim
HPC = NH // NCORES  # heads per core = 2
CW = HPC * DK  # output columns per core = 128
DC = D // 128  # D chunks = 8

_CACHE = {}
LAST_RESULTS = None


def _build(n_rows):
    """SPMD Bass program for one core. Each core computes batch-0 attention
    for its 2 heads (the reference only uses att[0]) and adds it to its
    column slice of tgt for all batches.

    tgt[0]/memory[0] arrive host-transposed ([D, N]) and pre-rounded to the
    fp32r grid, declared float32r, so DMA lands matmul-ready (fp32r runs at
    full PE rate for moving dim 512). Scores are computed transposed (k on
    partitions) so softmax's P feeds P.T@V with no P transposes; V carries
    an appended ones column so the same accumulation yields softmax row
    sums. K/Q/V live in per-512-row-group tiles so attention chunks can
    start as soon as their group's projections finish."""
    import concourse.mybir as mybir
    import concourse.tile as tile
    from concourse import bacc
    from concourse.masks import make_identity

    fp32 = mybir.dt.float32
    fp32r = mybir.dt.float32r
    bf16 = mybir.dt.bfloat16

    RT = n_rows // 128  # row tiles
    G = n_rows // 512  # 512-row groups
    QG = G
    KC = RT  # k chunks of 128

    nc = bacc.Bacc(None, target_bir_lowering=False)
    tgt0t = nc.declare_dram_parameter("tgt0t", [D, n_rows], fp32r, isOutput=False)
    mem0t = nc.declare_dram_parameter("mem0t", [D, n_rows], fp32r, isOutput=False)
    wqt = nc.declare_dram_parameter("wqt", [D, CW], fp32r, isOutput=False)
    wkt = nc.declare_dram_parameter("wkt", [D, CW], fp32r, isOutput=False)
    wvt = nc.declare_dram_parameter("wvt", [D, CW], fp32r, isOutput=False)
    tgtc = nc.declare_dram_parameter("tgtc", [B, n_rows, CW], fp32, isOutput=False)
    outc = nc.declare_dram_parameter("outc", [B, n_rows, CW], fp32, isOutput=True)

    Exp = mybir.ActivationFunctionType.Exp
    scale = 1.0 / np.sqrt(DK)

    with tile.TileContext(nc) as tc:
        with (
            tc.tile_pool(name="const", bufs=1) as const,
            tc.tile_pool(name="persist", bufs=1) as persist,
        ):
            ident = const.tile([128, 128], fp32)
            make_identity(nc, ident)

            # per-group K/Q/V tiles (fine-grained deps -> phase overlap)
            KT_gs = [
                persist.tile([128, 512], fp32r, tag=f"KT{g}", name=f"KT{g}")
                for g in range(G)
            ]
            QT_gs = [
                persist.tile([128, 512], fp32r, tag=f"QT{g}", name=f"QT{g}")
                for g in range(G)
            ]
            Vp_gs = [
                persist.tile([128, HPC, 4, DK + 1], bf16, tag=f"Vp{g}", name=f"Vp{g}")
                for g in range(G)
            ]
            att_sb = persist.tile([128, RT, CW], fp32, tag="att")
            tgtc_sb = persist.tile([128, B, RT, CW], fp32, tag="tgtc")

            ones_f32 = const.tile([128, HPC, 4], fp32, tag="ones")
            nc.vector.memset(ones_f32, 1.0)

            # ---- Phase A: loads + QKV projections (per 512-row group) ----
            with (
                tc.tile_pool(name="wst", bufs=1) as wst_pool,
                tc.tile_pool(name="grp", bufs=2) as grp_pool,
                tc.tile_pool(name="vtg", bufs=2) as vt_pool,
                tc.tile_pool(name="ps_w", bufs=1, space="PSUM") as ps_w,
                tc.tile_pool(name="ps_acc", bufs=2, space="PSUM") as ps_acc,
            ):
                # PE warmup during the initial DMA wait (HAM un-throttle)
                for _ in range(16):
                    pw = ps_w.tile([128, 128], fp32, tag="warm")
                    nc.tensor.transpose(pw, ident, ident)

                WTs = {}
                for name, w in (("q", wqt), ("k", wkt), ("v", wvt)):
                    wt = wst_pool.tile([128, DC, CW], fp32r, tag=f"wt{name}")
                    nc.sync.dma_start(
                        out=wt, in_=w[:, :].rearrange("(c p) q -> p c q", p=128)
                    )
                    WTs[name] = wt

                def emit_mem_group(g):
                    memT_g = grp_pool.tile(
                        [128, DC, 512], fp32r, tag="memTg", name=f"memT{g}"
                    )
                    for d in range(DC):
                        nc.sync.dma_start(
                            out=memT_g[:, d, :],
                            in_=mem0t[
                                d * 128 : (d + 1) * 128, g * 512 : (g + 1) * 512
                            ],
                        )
                    pk = ps_acc.tile([128, 512], fp32, tag="acc")
                    for d in range(DC):
                        nc.tensor.matmul(
                            pk, WTs["k"][:, d, :], memT_g[:, d, :],
                            start=(d == 0), stop=(d == DC - 1),
                        )
                    nc.vector.tensor_copy(out=KT_gs[g], in_=pk)
                    pv = ps_acc.tile([128, 512], fp32, tag="acc")
                    for d in range(DC):
                        nc.tensor.matmul(
                            pv, WTs["v"][:, d, :], memT_g[:, d, :],
                            start=(d == 0), stop=(d == DC - 1),
                        )
                    vt_g = vt_pool.tile([128, 512], fp32, tag="vtg")
                    nc.vector.tensor_copy(out=vt_g, in_=pv)
                    for t in range(4):
                        ptr = ps_w.tile([128, 128], fp32, tag="warm")
                        nc.tensor.transpose(ptr, vt_g[:, t * 128 : (t + 1) * 128], ident)
                        nc.vector.tensor_copy(
                            out=Vp_gs[g][:, 0, t, 0:DK], in_=ptr[:, 0:DK]
                        )
                        nc.vector.tensor_copy(
                            out=Vp_gs[g][:, 1, t, 0:DK], in_=ptr[:, DK : 2 * DK]
                        )
                    nc.vector.tensor_copy(out=Vp_gs[g][:, :, :, DK], in_=ones_f32)

                def emit_tgt_group(g):
                    tgtT_g = grp_pool.tile(
                        [128, DC, 512], fp32r, tag="tgtTg", name=f"tgtT{g}"
                    )
                    for d in range(DC):
                        nc.sync.dma_start(
                            out=tgtT_g[:, d, :],
                            in_=tgt0t[
                                d * 128 : (d + 1) * 128, g * 512 : (g + 1) * 512
                            ],
                        )
                    pq = ps_acc.tile([128, 512], fp32, tag="acc")
                    for d in range(DC):
                        nc.tensor.matmul(
                            pq, WTs["q"][:, d, :], tgtT_g[:, d, :],
                            start=(d == 0), stop=(d == DC - 1),
                        )
                    nc.vector.tensor_copy(out=QT_gs[g], in_=pq)


                # ---- Phase B: attention per q-group, heads paired ----
                with (
                    tc.tile_pool(name="pt", bufs=1) as pt_pool,
                    tc.tile_pool(name="usb", bufs=2) as usb_pool,
                    tc.tile_pool(name="small", bufs=8) as small_pool,
                    tc.tile_pool(name="ps_st", bufs=2, space="PSUM") as ps_st,
                    tc.tile_pool(name="ps_u", bufs=1, space="PSUM") as ps_u,
                ):
                    def emit_st_block(qg, pts, jp):
                        # one exp pair: chunks 2*jp, 2*jp+1 for both heads
                        psts = [
                            ps_st.tile(
                                [128, 2, 512], fp32, tag="st", name=f"st{qg}_{jp}_{h}"
                            )
                            for h in range(HPC)
                        ]
                        for jj in range(2):
                            j = jp * 2 + jj
                            kg, kt = j // 4, j % 4
                            for h in range(HPC):
                                hs = h * DK
                                nc.tensor.matmul(
                                    psts[h][:, jj, :],
                                    KT_gs[kg][hs : hs + DK, kt * 128 : (kt + 1) * 128],
                                    QT_gs[qg][hs : hs + DK, :],
                                    start=True, stop=True,
                                )
                        for h in range(HPC):
                            nc.scalar.activation(
                                out=pts[h][:, jp * 2 : jp * 2 + 2, :],
                                in_=psts[h],
                                func=Exp,
                                scale=float(scale),
                            )

                    all_pts = {}
                    # ramp: stream qg0's score chunks between memory groups so
                    # ScalarE starts exp work as early as possible
                    emit_tgt_group(0)
                    all_pts[0] = [
                        pt_pool.tile(
                            [128, KC, 512], bf16, tag=f"pt{h}", name=f"pt{h}_0"
                        )
                        for h in range(HPC)
                    ]
                    for g in range(G):
                        emit_mem_group(g)
                        for jp in range(2 * g, 2 * g + 2):
                            emit_st_block(0, all_pts[0], jp)

                    for b in range(B):
                        nc.sync.dma_start(
                            out=tgtc_sb[:, b, :, :],
                            in_=tgtc[b, :, :].rearrange("(t p) c -> p t c", p=128),
                        )

                    for qg in range(QG):
                        if qg + 1 < QG:
                            emit_tgt_group(qg + 1)
                            all_pts[qg + 1] = [
                                pt_pool.tile(
                                    [128, KC, 512], bf16, tag=f"pt{h}",
                                    name=f"pt{h}_{qg+1}",
                                )
                                for h in range(HPC)
                            ]
                        qsl = slice(qg * 512, (qg + 1) * 512)
                        pts = all_pts[qg]
                        if qg > 0:
                            for jp in range(KC // 2):
                                emit_st_block(qg, pts, jp)
                        for h in range(HPC):
                            hs = h * DK
                            pu = ps_u.tile([DK + 1, 512], fp32, tag="u")
                            for j in range(KC):
                                nc.tensor.matmul(
                                    pu,
                                    Vp_gs[j // 4][:, h, j % 4, :],
                                    pts[h][:, j, :],
                                    start=(j == 0), stop=(j == KC - 1),
                                )
                            pu_sb = usb_pool.tile([DK + 1, 512], fp32, tag="usb")
                            nc.vector.tensor_copy(out=pu_sb, in_=pu)
                            for s in range(4):
                                pat = ps_w.tile([128, 128], fp32, tag="warm")
                                nc.tensor.transpose(
                                    pat[:, 0 : DK + 1],
                                    pu_sb[:, s * 128 : (s + 1) * 128],
                                    ident[0 : DK + 1, 0 : DK + 1],
                                )
                                rec = small_pool.tile([128, 1], fp32, tag="rec")
                                nc.vector.reciprocal(rec, pat[:, DK : DK + 1])
                                nc.vector.tensor_scalar_mul(
                                    att_sb[:, qg * 4 + s, hs : hs + DK],
                                    in0=pat[:, 0:DK],
                                    scalar1=rec,
                                )
                        # final broadcast add + store for this q-group's rows
                        for b in range(B):
                            nc.vector.tensor_add(
                                out=tgtc_sb[:, b, qg * 4 : (qg + 1) * 4, :],
                                in0=tgtc_sb[:, b, qg * 4 : (qg + 1) * 4, :],
                                in1=att_sb[:, qg * 4 : (qg + 1) * 4, :],
                            )
                            nc.sync.dma_start(
                                out=outc[b, qsl, :].rearrange(
                                    "(t p) c -> p t c", p=128
                                ),
                                in_=tgtc_sb[:, b, qg * 4 : (qg + 1) * 4, :],
                            )

    nc.finalize()
    return nc


def _get_nc(n_rows):
    if n_rows not in _CACHE:
        _CACHE[n_rows] = _build(n_rows)
    return _CACHE[n_rows]


def _round_fp32r(x):
    """Round fp32 to the fp32r grid (11 explicit mantissa bits, RNE)."""
    v = np.ascontiguousarray(x, dtype=np.float32).view(np.uint32)
    lo = v & np.uint32(0xFFF)
    base = v & ~np.uint32(0xFFF)
    lsb = (v >> np.uint32(12)) & np.uint32(1)
    up = (lo > 0x800) | ((lo == 0x800) & (lsb == 1))
    out = base + (up.astype(np.uint32) << np.uint32(12))
    return out.view(np.float32)


def _run(tgt, memory, Wq, Wk, Wv, trace=False):
    global LAST_RESULTS
    from concourse.bass_utils import run_bass_kernel_spmd

    n_rows = tgt.shape[1]
    nc = _get_nc(n_rows)

    tgt = np.ascontiguousarray(tgt, dtype=np.float32)
    memory = np.ascontiguousarray(memory, dtype=np.float32)
    tgt0t = _round_fp32r(np.ascontiguousarray(tgt[0].T))
    mem0t = _round_fp32r(np.ascontiguousarray(memory[0].T))

    in_maps = []
    for c in range(NCORES):
        sl = slice(c * CW, (c + 1) * CW)
        in_maps.append(
            {
                "tgt0t": tgt0t,
                "mem0t": mem0t,
                "wqt": _round_fp32r(Wq[sl, :].T),
                "wkt": _round_fp32r(Wk[sl, :].T),
                "wvt": _round_fp32r(Wv[sl, :].T),
                "tgtc": np.ascontiguousarray(tgt[:, :, sl]),
            }
        )
    res = run_bass_kernel_spmd(nc, in_maps, list(range(NCORES)), trace=trace)
    LAST_RESULTS = res
    out = np.concatenate([res.results[c]["outc"] for c in range(NCORES)], axis=2)
    return out


def kernel(tgt, memory, Wq, Wk, Wv):
    return _run(tgt, memory, Wq, Wk, Wv)



# revision 10
# speedup vs baseline: 1.4003x; 1.4003x over previous
import sys

sys.path.insert(0, "/opt/trn_rl_repo")

import numpy as np

NCORES = 8
B, FULL_N, D = 4, 2048, 1024
NH = 16
DK = 64  # head dim
HPC = NH // NCORES  # heads per core = 2
CW = HPC * DK  # output columns per core = 128
DC = D // 128  # D chunks = 8
WSCALE = 16.0  # host-side weight scale so fp8 weights sit in normal range
PSCALE = 0.25  # exp output scale: keeps max prob value < fp8e4 max (240)

_CACHE = {}
LAST_RESULTS = None


def _build(n_rows):
    """SPMD Bass program for one core. Each core computes batch-0 attention
    for its 2 heads (the reference only uses att[0]) and adds it to its
    column slice of tgt for all batches.

    v2: fp8(e4m3) everywhere the precision budget allows. QKV projections
    run as fp8 DoubleRow matmuls (2 contraction chunks per instruction),
    probs are written by ScalarE exp directly as fp8 and consumed by fp8
    DoubleRow P.T@V matmuls whose stationary V carries a x16 ones column
    (row sums land in PSUM row 64 of the same accumulation). Scores stay
    bf16 with the two heads on disjoint PE row groups (partitions 0-63 vs
    64-127) so their matmuls run tile-concurrent. Residual adds and output
    are bf16 (DVE 2x/4x modes); the host upcasts."""
    import concourse.mybir as mybir
    import concourse.tile as tile
    from concourse import bacc
    from concourse.masks import make_identity

    fp32 = mybir.dt.float32
    bf16 = mybir.dt.bfloat16
    fp8 = mybir.dt.float8e4
    DR = mybir.MatmulPerfMode.DoubleRow

    RT = n_rows // 128  # row tiles
    G = n_rows // 512  # 512-row groups
    QG = G
    KC = RT  # k chunks of 128

    nc = bacc.Bacc(None, target_bir_lowering=False)
    tgt0t = nc.declare_dram_parameter("tgt0t", [D, n_rows], fp8, isOutput=False)
    mem0t = nc.declare_dram_parameter("mem0t", [D, n_rows], fp8, isOutput=False)
    wqt = nc.declare_dram_parameter("wqt", [D, CW], fp8, isOutput=False)
    wkt = nc.declare_dram_parameter("wkt", [D, CW], fp8, isOutput=False)
    wvt = nc.declare_dram_parameter("wvt", [D, CW], fp8, isOutput=False)
    tgtc = nc.declare_dram_parameter("tgtc", [B, n_rows, CW], bf16, isOutput=False)
    outc = nc.declare_dram_parameter("outc", [B, n_rows, CW], bf16, isOutput=True)

    Exp = mybir.ActivationFunctionType.Exp
    # Wq,Wk both carry x16 -> scores in PSUM are 256x; fold into exp scale.
    scale = 1.0 / (np.sqrt(DK) * WSCALE * WSCALE)
    # Schraudolph exp-as-bits constants (bf16 = top half of fp32):
    # i16 = round(A*s + B); bits(i16) ~= exp(s*scale)
    SCH_A = float(128.0 * np.log2(np.e) * scale)
    SCH_B = float(128.0 * (127.0 - 0.0436775))
    DVE_EXP_JP = (2, 5)

    with tile.TileContext(nc) as tc:
        with (
            tc.tile_pool(name="const", bufs=1) as const,
            tc.tile_pool(name="persist", bufs=1) as persist,
        ):
            identb = const.tile([128, 128], bf16)
            make_identity(nc, identb)

            # per-group K/Q/V tiles (fine-grained deps -> phase overlap)
            KT_gs = [
                persist.tile([128, 512], bf16, tag=f"KT{g}", name=f"KT{g}")
                for g in range(G)
            ]
            QT_gs = [
                persist.tile([128, 512], bf16, tag=f"QT{g}", name=f"QT{g}")
                for g in range(G)
            ]
            # bf16 V, transposed: [k-row%128, head, k-chunk-in-group, 65]
            # cols 0:64 = 16*v, col 64 = 16.0 (ones column -> row sums)
            Vp_gs = [
                persist.tile([128, HPC, 4, DK + 1], bf16, tag=f"Vp{g}", name=f"Vp{g}")
                for g in range(G)
            ]
            att_sb = persist.tile([128, RT, CW], bf16, tag="att")
            tgtc_sb = persist.tile([128, B, RT, CW], bf16, tag="tgtc")

            # ---- Phase A: loads + QKV projections (per 512-row group) ----
            with (
                tc.tile_pool(name="wst", bufs=1) as wst_pool,
                tc.tile_pool(name="grp", bufs=2) as grp_pool,
                tc.tile_pool(name="vtg", bufs=2) as vt_pool,
                tc.tile_pool(name="ps_w", bufs=1, space="PSUM") as ps_w,
                tc.tile_pool(name="ps_acc", bufs=2, space="PSUM") as ps_acc,
            ):
                # PE warmup during the initial DMA wait. Real matmuls,
                # not transposes: transpose-mode does not count as PE-busy
                # for the HAM clock gate, so only genuine MMs un-throttle.
                for _ in range(14):
                    pw = ps_acc.tile([128, 512], fp32, tag="acc")
                    nc.tensor.matmul(
                        pw[:, 0:128], identb, identb, start=True, stop=True
                    )

                WTs = {}
                for name, w in (("q", wqt), ("k", wkt), ("v", wvt)):
                    wt = wst_pool.tile([128, DC, CW], fp8, tag=f"wt{name}")
                    nc.sync.dma_start(
                        out=wt, in_=w[:, :].rearrange("(c p) q -> p c q", p=128)
                    )
                    WTs[name] = wt

                def emit_mem_group(g):
                    memT_g = grp_pool.tile(
                        [128, DC, 512], fp8, tag="memTg", name=f"memT{g}"
                    )
                    nc.sync.dma_start(
                        out=memT_g,
                        in_=mem0t[:, g * 512 : (g + 1) * 512].rearrange(
                            "(c p) n -> p c n", p=128
                        ),
                    )
                    pk = ps_acc.tile([128, 512], fp32, tag="acc")
                    for d in range(DC // 2):
                        nc.tensor.matmul(
                            pk,
                            WTs["k"][:, 2 * d : 2 * d + 2, :],
                            memT_g[:, 2 * d : 2 * d + 2, :],
                            start=(d == 0), stop=(d == DC // 2 - 1),
                            perf_mode=DR,
                        )
                    nc.vector.tensor_copy(out=KT_gs[g], in_=pk)
                    pv = ps_acc.tile([128, 512], fp32, tag="acc")
                    for d in range(DC // 2):
                        nc.tensor.matmul(
                            pv,
                            WTs["v"][:, 2 * d : 2 * d + 2, :],
                            memT_g[:, 2 * d : 2 * d + 2, :],
                            start=(d == 0), stop=(d == DC // 2 - 1),
                            perf_mode=DR,
                        )
                    vt_g = vt_pool.tile([128, 512], bf16, tag="vtg")
                    nc.vector.tensor_copy(out=vt_g, in_=pv)
                    nc.vector.memset(Vp_gs[g], 16.0)
                    for t in range(4):
                        ptr = ps_w.tile([128, 128], bf16, tag="warm")
                        nc.tensor.transpose(
                            ptr, vt_g[:, t * 128 : (t + 1) * 128], identb
                        )
                        nc.vector.tensor_copy(
                            out=Vp_gs[g][:, 0, t, 0:DK], in_=ptr[:, 0:DK]
                        )
                        nc.vector.tensor_copy(
                            out=Vp_gs[g][:, 1, t, 0:DK], in_=ptr[:, DK : 2 * DK]
                        )

                def emit_tgt_group(g):
                    tgtT_g = grp_pool.tile(
                        [128, DC, 512], fp8, tag="tgtTg", name=f"tgtT{g}"
                    )
                    nc.sync.dma_start(
                        out=tgtT_g,
                        in_=tgt0t[:, g * 512 : (g + 1) * 512].rearrange(
                            "(c p) n -> p c n", p=128
                        ),
                    )
                    pq = ps_acc.tile([128, 512], fp32, tag="acc")
                    for d in range(DC // 2):
                        nc.tensor.matmul(
                            pq,
                            WTs["q"][:, 2 * d : 2 * d + 2, :],
                            tgtT_g[:, 2 * d : 2 * d + 2, :],
                            start=(d == 0), stop=(d == DC // 2 - 1),
                            perf_mode=DR,
                        )
                    nc.vector.tensor_copy(out=QT_gs[g], in_=pq)

                # ---- Phase B: attention per q-group, heads paired ----
                with (
                    tc.tile_pool(name="pt", bufs=1) as pt_pool,
                    tc.tile_pool(name="usb", bufs=2) as usb_pool,
                    tc.tile_pool(name="small", bufs=8) as small_pool,
                    tc.tile_pool(name="ps_st", bufs=2, space="PSUM") as ps_st,
                    tc.tile_pool(name="ps_u", bufs=1, space="PSUM") as ps_u,
                ):
                    def emit_st_block(qg, pts, jp):
                        # one exp pair: chunks 2*jp, 2*jp+1 for both heads;
                        # heads interleaved so their matmuls hit disjoint PE
                        # row groups (partitions 0-63 / 64-127) concurrently
                        psts = [
                            ps_st.tile(
                                [128, 2, 512], fp32, tag="st", name=f"st{qg}_{jp}_{h}"
                            )
                            for h in range(HPC)
                        ]
                        for jj in range(2):
                            j = jp * 2 + jj
                            kg, kt = j // 4, j % 4
                            for h in range(HPC):
                                hs = h * DK
                                nc.tensor.matmul(
                                    psts[h][:, jj, :],
                                    KT_gs[kg][hs : hs + DK, kt * 128 : (kt + 1) * 128],
                                    QT_gs[qg][hs : hs + DK, :],
                                    start=True, stop=True,
                                )
                        for h in range(HPC):
                            o = pts[h][:, jp * 2 : jp * 2 + 2, :]
                            if jp in DVE_EXP_JP:
                                # Schraudolph exp on DVE: bf16 bit pattern via
                                # int16 affine + bitcast; +-3% per element,
                                # absorbed by the softmax normalization
                                nc.vector.tensor_scalar(
                                    out=o.bitcast(mybir.dt.int16),
                                    in0=psts[h],
                                    scalar1=SCH_A,
                                    scalar2=SCH_B,
                                    op0=mybir.AluOpType.mult,
                                    op1=mybir.AluOpType.add,
                                )
                            else:
                                nc.scalar.activation(
                                    out=o,
                                    in_=psts[h],
                                    func=Exp,
                                    scale=float(scale),
                                )

                    all_pts = {}
                    # ramp: stream qg0's score chunks between memory groups so
                    # ScalarE starts exp work as early as possible
                    emit_tgt_group(0)
                    all_pts[0] = [
                        pt_pool.tile(
                            [128, KC, 512], bf16, tag=f"pt{h}", name=f"pt{h}_0"
                        )
                        for h in range(HPC)
                    ]
                    for g in range(G):
                        emit_mem_group(g)
                        for jp in range(2 * g, 2 * g + 2):
                            emit_st_block(0, all_pts[0], jp)

                    for b in range(B):
                        nc.sync.dma_start(
                            out=tgtc_sb[:, b, :, :],
                            in_=tgtc[b, :, :].rearrange("(t p) c -> p t c", p=128),
                        )

                    for qg in range(QG):
                        if qg + 1 < QG:
                            emit_tgt_group(qg + 1)
                            all_pts[qg + 1] = [
                                pt_pool.tile(
                                    [128, KC, 512], bf16, tag=f"pt{h}",
                                    name=f"pt{h}_{qg+1}",
                                )
                                for h in range(HPC)
                            ]
                        qsl = slice(qg * 512, (qg + 1) * 512)
                        pts = all_pts[qg]
                        if qg > 0:
                            for jp in range(KC // 2):
                                emit_st_block(qg, pts, jp)
                        for h in range(HPC):
                            hs = h * DK
                            pu = ps_u.tile([DK + 1, 512], fp32, tag="u")
                            for j in range(KC):
                                nc.tensor.matmul(
                                    pu,
                                    Vp_gs[j // 4][:, h, j % 4, :],
                                    pts[h][:, j, :],
                                    start=(j == 0), stop=(j == KC - 1),
                                )
                            pu_sb = usb_pool.tile([DK + 1, 512], bf16, tag="usb")
                            nc.vector.tensor_copy(out=pu_sb, in_=pu)
                            for s in range(4):
                                pat = ps_w.tile([128, 128], bf16, tag="warm")
                                nc.tensor.transpose(
                                    pat[:, 0 : DK + 1],
                                    pu_sb[:, s * 128 : (s + 1) * 128],
                                    identb[0 : DK + 1, 0 : DK + 1],
                                )
                                pat_sb = small_pool.tile(
                                    [128, DK + 1], bf16, tag="patsb"
                                )
                                nc.vector.tensor_copy(out=pat_sb, in_=pat[:, 0 : DK + 1])
                                rec = small_pool.tile([128, 1], fp32, tag="rec")
                                nc.vector.reciprocal(rec, pat_sb[:, DK : DK + 1])
                                nc.gpsimd.tensor_scalar_mul(
                                    att_sb[:, qg * 4 + s, hs : hs + DK],
                                    in0=pat_sb[:, 0:DK],
                                    scalar1=rec,
                                )
                        # final broadcast add + store for this q-group's rows
                        for b in range(B):
                            nc.gpsimd.tensor_add(
                                out=tgtc_sb[:, b, qg * 4 : (qg + 1) * 4, :],
                                in0=tgtc_sb[:, b, qg * 4 : (qg + 1) * 4, :],
                                in1=att_sb[:, qg * 4 : (qg + 1) * 4, :],
                            )
                            nc.sync.dma_start(
                                out=outc[b, qsl, :].rearrange(
                                    "(t p) c -> p t c", p=128
                                ),
                                in_=tgtc_sb[:, b, qg * 4 : (qg + 1) * 4, :],
                            )

    nc.finalize()
    return nc


def _get_nc(n_rows):
    if n_rows not in _CACHE:
        _CACHE[n_rows] = _build(n_rows)
    return _CACHE[n_rows]


def _to_fp8(x):
    import ml_dtypes

    return np.ascontiguousarray(x, dtype=np.float32).astype(ml_dtypes.float8_e4m3)


def _to_bf16(x):
    import ml_dtypes

    return np.ascontiguousarray(x, dtype=np.float32).astype(ml_dtypes.bfloat16)


def _run(tgt, memory, Wq, Wk, Wv, trace=False):
    global LAST_RESULTS
    from concourse.bass_utils import run_bass_kernel_spmd

    n_rows = tgt.shape[1]
    nc = _get_nc(n_rows)

    tgt = np.ascontiguousarray(tgt, dtype=np.float32)
    memory = np.ascontiguousarray(memory, dtype=np.float32)
    tgt0t = _to_fp8(tgt[0].T)
    mem0t = _to_fp8(memory[0].T)

    in_maps = []
    for c in range(NCORES):
        sl = slice(c * CW, (c + 1) * CW)
        in_maps.append(
            {
                "tgt0t": tgt0t,
                "mem0t": mem0t,
                "wqt": _to_fp8(Wq[sl, :].T * WSCALE),
                "wkt": _to_fp8(Wk[sl, :].T * WSCALE),
                "wvt": _to_fp8(Wv[sl, :].T * WSCALE),
                "tgtc": _to_bf16(tgt[:, :, sl]),
            }
        )
    res = run_bass_kernel_spmd(nc, in_maps, list(range(NCORES)), trace=trace)
    LAST_RESULTS = res
    out = np.concatenate(
        [res.results[c]["outc"].astype(np.float32) for c in range(NCORES)], axis=2
    )
    return out


def kernel(tgt, memory, Wq, Wk, Wv):
    return _run(tgt, memory, Wq, Wk, Wv)


# revision 11
# speedup vs baseline: 1.4969x; 1.0690x over previous
import sys

sys.path.insert(0, "/opt/trn_rl_repo")

import numpy as np

NCORES = 8
B, FULL_N, D = 4, 2048, 1024
NH = 16
DK = 64  # head dim
HPC = NH // NCORES  # heads per core = 2
CW = HPC * DK  # output columns per core = 128
DC = D // 128  # D chunks = 8
WSCALE = 16.0  # host-side weight scale so fp8 weights sit in normal range
PSCALE = 0.25  # exp output scale: keeps max prob value < fp8e4 max (240)

_CACHE = {}
LAST_RESULTS = None


def _build(n_rows):
    """SPMD Bass program for one core. Each core computes batch-0 attention
    for its 2 heads (the reference only uses att[0]) and adds it to its
    column slice of tgt for all batches.

    v2: fp8(e4m3) everywhere the precision budget allows. QKV projections
    run as fp8 DoubleRow matmuls (2 contraction chunks per instruction),
    probs are written by ScalarE exp directly as fp8 and consumed by fp8
    DoubleRow P.T@V matmuls whose stationary V carries a x16 ones column
    (row sums land in PSUM row 64 of the same accumulation). Scores stay
    bf16 with the two heads on disjoint PE row groups (partitions 0-63 vs
    64-127) so their matmuls run tile-concurrent. Residual adds and output
    are bf16 (DVE 2x/4x modes); the host upcasts."""
    import concourse.mybir as mybir
    import concourse.tile as tile
    from concourse import bacc
    from concourse.masks import make_identity

    fp32 = mybir.dt.float32
    bf16 = mybir.dt.bfloat16
    fp8 = mybir.dt.float8e4
    DR = mybir.MatmulPerfMode.DoubleRow

    RT = n_rows // 128  # row tiles
    G = n_rows // 512  # 512-row groups
    QG = G
    KC = RT  # k chunks of 128

    nc = bacc.Bacc(None, target_bir_lowering=False)
    tgt0t = nc.declare_dram_parameter("tgt0t", [D, n_rows], fp8, isOutput=False)
    mem0t = nc.declare_dram_parameter("mem0t", [D, n_rows], fp8, isOutput=False)
    wqt = nc.declare_dram_parameter("wqt", [D, CW], fp8, isOutput=False)
    wkt = nc.declare_dram_parameter("wkt", [D, CW], fp8, isOutput=False)
    wvt = nc.declare_dram_parameter("wvt", [D, CW], fp8, isOutput=False)
    tgtc = nc.declare_dram_parameter("tgtc", [B, n_rows, CW], bf16, isOutput=False)
    outc = nc.declare_dram_parameter("outc", [B, n_rows, CW], bf16, isOutput=True)

    Exp = mybir.ActivationFunctionType.Exp
    # Wq,Wk both carry x16 -> scores in PSUM are 256x; fold into exp scale.
    scale = 1.0 / (np.sqrt(DK) * WSCALE * WSCALE)
    # Schraudolph exp-as-bits constants (bf16 = top half of fp32):
    # i16 = round(A*s + B); bits(i16) ~= exp(s*scale)
    SCH_A = float(128.0 * np.log2(np.e) * scale)
    SCH_B = float(128.0 * (127.0 - 0.0436775))
    DVE_EXP_JP = (2, 5)

    with tile.TileContext(nc) as tc:
        with (
            tc.tile_pool(name="const", bufs=1) as const,
            tc.tile_pool(name="persist", bufs=1) as persist,
        ):
            identb = const.tile([128, 128], bf16)
            make_identity(nc, identb)

            # per-group K/Q/V tiles (fine-grained deps -> phase overlap)
            KT_gs = [
                persist.tile([128, 512], bf16, tag=f"KT{g}", name=f"KT{g}")
                for g in range(G)
            ]
            QT_gs = [
                persist.tile([128, 512], bf16, tag=f"QT{g}", name=f"QT{g}")
                for g in range(G)
            ]
            # bf16 V, transposed: [k-row%128, head, k-chunk-in-group, 65]
            # cols 0:64 = 16*v, col 64 = 16.0 (ones column -> row sums)
            Vp_gs = [
                persist.tile([128, HPC, 4, DK + 1], bf16, tag=f"Vp{g}", name=f"Vp{g}")
                for g in range(G)
            ]
            att_sb = persist.tile([128, RT, CW], bf16, tag="att")
            tgtc_sb = persist.tile([128, B, RT, CW], bf16, tag="tgtc")

            # ---- Phase A: loads + QKV projections (per 512-row group) ----
            with (
                tc.tile_pool(name="wst", bufs=1) as wst_pool,
                tc.tile_pool(name="grp", bufs=2) as grp_pool,
                tc.tile_pool(name="vtg", bufs=2) as vt_pool,
                tc.tile_pool(name="ps_w", bufs=1, space="PSUM") as ps_w,
                tc.tile_pool(name="ps_acc", bufs=2, space="PSUM") as ps_acc,
            ):
                # PE warmup during the initial DMA wait. Real matmuls,
                # not transposes: transpose-mode does not count as PE-busy
                # for the HAM clock gate, so only genuine MMs un-throttle.
                for _ in range(24):
                    pw = ps_acc.tile([128, 512], fp32, tag="acc")
                    nc.tensor.matmul(
                        pw[:, 0:128], identb, identb, start=True, stop=True
                    )

                WTs = {}
                for name, w in (("q", wqt), ("k", wkt), ("v", wvt)):
                    wt = wst_pool.tile([128, DC, CW], fp8, tag=f"wt{name}")
                    nc.sync.dma_start(
                        out=wt, in_=w[:, :].rearrange("(c p) q -> p c q", p=128)
                    )
                    WTs[name] = wt

                def emit_mem_group(g):
                    memT_g = grp_pool.tile(
                        [128, DC, 512], fp8, tag="memTg", name=f"memT{g}"
                    )
                    for half in range(2):
                        nc.sync.dma_start(
                            out=memT_g[:, 4 * half : 4 * half + 4, :],
                            in_=mem0t[
                                512 * half : 512 * half + 512,
                                g * 512 : (g + 1) * 512,
                            ].rearrange("(c p) n -> p c n", p=128),
                        )
                    pk = ps_acc.tile([128, 512], fp32, tag="acc")
                    for d in range(DC // 2):
                        nc.tensor.matmul(
                            pk,
                            WTs["k"][:, 2 * d : 2 * d + 2, :],
                            memT_g[:, 2 * d : 2 * d + 2, :],
                            start=(d == 0), stop=(d == DC // 2 - 1),
                            perf_mode=DR,
                        )
                    nc.vector.tensor_copy(out=KT_gs[g], in_=pk)
                    pv = ps_acc.tile([128, 512], fp32, tag="acc")
                    for d in range(DC // 2):
                        nc.tensor.matmul(
                            pv,
                            WTs["v"][:, 2 * d : 2 * d + 2, :],
                            memT_g[:, 2 * d : 2 * d + 2, :],
                            start=(d == 0), stop=(d == DC // 2 - 1),
                            perf_mode=DR,
                        )
                    vt_g = vt_pool.tile([128, 512], bf16, tag="vtg")
                    nc.vector.tensor_copy(out=vt_g, in_=pv)
                    nc.vector.memset(Vp_gs[g], 16.0)
                    for t in range(4):
                        ptr = ps_w.tile([128, 128], bf16, tag="warm")
                        nc.tensor.transpose(
                            ptr, vt_g[:, t * 128 : (t + 1) * 128], identb
                        )
                        nc.vector.tensor_copy(
                            out=Vp_gs[g][:, 0, t, 0:DK], in_=ptr[:, 0:DK]
                        )
                        nc.vector.tensor_copy(
                            out=Vp_gs[g][:, 1, t, 0:DK], in_=ptr[:, DK : 2 * DK]
                        )

                def emit_tgt_group(g):
                    tgtT_g = grp_pool.tile(
                        [128, DC, 512], fp8, tag="tgtTg", name=f"tgtT{g}"
                    )
                    for half in range(2):
                        nc.sync.dma_start(
                            out=tgtT_g[:, 4 * half : 4 * half + 4, :],
                            in_=tgt0t[
                                512 * half : 512 * half + 512,
                                g * 512 : (g + 1) * 512,
                            ].rearrange("(c p) n -> p c n", p=128),
                        )
                    pq = ps_acc.tile([128, 512], fp32, tag="acc")
                    for d in range(DC // 2):
                        nc.tensor.matmul(
                            pq,
                            WTs["q"][:, 2 * d : 2 * d + 2, :],
                            tgtT_g[:, 2 * d : 2 * d + 2, :],
                            start=(d == 0), stop=(d == DC // 2 - 1),
                            perf_mode=DR,
                        )
                    nc.vector.tensor_copy(out=QT_gs[g], in_=pq)

                # ---- Phase B: attention per q-group, heads paired ----
                with (
                    tc.tile_pool(name="pt", bufs=1) as pt_pool,
                    tc.tile_pool(name="usb", bufs=2) as usb_pool,
                    tc.tile_pool(name="small", bufs=8) as small_pool,
                    tc.tile_pool(name="ps_st", bufs=2, space="PSUM") as ps_st,
                    tc.tile_pool(name="ps_u", bufs=1, space="PSUM") as ps_u,
                ):
                    def emit_st_block(qg, pts, jp):
                        # one exp pair: chunks 2*jp, 2*jp+1 for both heads;
                        # heads interleaved so their matmuls hit disjoint PE
                        # row groups (partitions 0-63 / 64-127) concurrently
                        psts = [
                            ps_st.tile(
                                [128, 2, 512], fp32, tag="st", name=f"st{qg}_{jp}_{h}"
                            )
                            for h in range(HPC)
                        ]
                        for jj in range(2):
                            j = jp * 2 + jj
                            kg, kt = j // 4, j % 4
                            for h in range(HPC):
                                hs = h * DK
                                nc.tensor.matmul(
                                    psts[h][:, jj, :],
                                    KT_gs[kg][hs : hs + DK, kt * 128 : (kt + 1) * 128],
                                    QT_gs[qg][hs : hs + DK, :],
                                    start=True, stop=True,
                                )
                        for h in range(HPC):
                            o = pts[h][:, jp * 2 : jp * 2 + 2, :]
                            if jp in DVE_EXP_JP:
                                # Schraudolph exp on DVE: bf16 bit pattern via
                                # int16 affine + bitcast; +-3% per element,
                                # absorbed by the softmax normalization
                                nc.vector.tensor_scalar(
                                    out=o.bitcast(mybir.dt.int16),
                                    in0=psts[h],
                                    scalar1=SCH_A,
                                    scalar2=SCH_B,
                                    op0=mybir.AluOpType.mult,
                                    op1=mybir.AluOpType.add,
                                )
                            else:
                                nc.scalar.activation(
                                    out=o,
                                    in_=psts[h],
                                    func=Exp,
                                    scale=float(scale),
                                )

                    all_pts = {}
                    # ramp: stream qg0's score chunks between memory groups so
                    # ScalarE starts exp work as early as possible
                    emit_mem_group(0)
                    emit_tgt_group(0)
                    all_pts[0] = [
                        pt_pool.tile(
                            [128, KC, 512], bf16, tag=f"pt{h}", name=f"pt{h}_0"
                        )
                        for h in range(HPC)
                    ]
                    for jp in range(2):
                        emit_st_block(0, all_pts[0], jp)
                    for g in range(1, G):
                        emit_mem_group(g)
                        for jp in range(2 * g, 2 * g + 2):
                            emit_st_block(0, all_pts[0], jp)

                    for b in range(B):
                        nc.sync.dma_start(
                            out=tgtc_sb[:, b, :, :],
                            in_=tgtc[b, :, :].rearrange("(t p) c -> p t c", p=128),
                        )

                    for qg in range(QG):
                        if qg + 1 < QG:
                            emit_tgt_group(qg + 1)
                            all_pts[qg + 1] = [
                                pt_pool.tile(
                                    [128, KC, 512], bf16, tag=f"pt{h}",
                                    name=f"pt{h}_{qg+1}",
                                )
                                for h in range(HPC)
                            ]
                        qsl = slice(qg * 512, (qg + 1) * 512)
                        pts = all_pts[qg]
                        if qg > 0:
                            for jp in range(KC // 2):
                                emit_st_block(qg, pts, jp)
                        for h in range(HPC):
                            hs = h * DK
                            pu = ps_u.tile([DK + 1, 512], fp32, tag="u")
                            for j in range(KC):
                                nc.tensor.matmul(
                                    pu,
                                    Vp_gs[j // 4][:, h, j % 4, :],
                                    pts[h][:, j, :],
                                    start=(j == 0), stop=(j == KC - 1),
                                )
                            pu_sb = usb_pool.tile([DK + 1, 512], bf16, tag="usb")
                            nc.vector.tensor_copy(out=pu_sb, in_=pu)
                            for s in range(4):
                                pat = ps_w.tile([128, 128], bf16, tag="warm")
                                nc.tensor.transpose(
                                    pat[:, 0 : DK + 1],
                                    pu_sb[:, s * 128 : (s + 1) * 128],
                                    identb[0 : DK + 1, 0 : DK + 1],
                                )
                                rec = small_pool.tile([128, 1], fp32, tag="rec")
                                nc.vector.reciprocal(rec, pat[:, DK : DK + 1])
                                nc.vector.tensor_scalar_mul(
                                    att_sb[:, qg * 4 + s, hs : hs + DK],
                                    in0=pat[:, 0:DK],
                                    scalar1=rec,
                                )
                        # final broadcast add + store for this q-group's rows
                        for b in range(B):
                            nc.vector.tensor_add(
                                out=tgtc_sb[:, b, qg * 4 : (qg + 1) * 4, :],
                                in0=tgtc_sb[:, b, qg * 4 : (qg + 1) * 4, :],
                                in1=att_sb[:, qg * 4 : (qg + 1) * 4, :],
                            )
                            nc.sync.dma_start(
                                out=outc[b, qsl, :].rearrange(
                                    "(t p) c -> p t c", p=128
                                ),
                                in_=tgtc_sb[:, b, qg * 4 : (qg + 1) * 4, :],
                            )

    nc.finalize()
    return nc


def _get_nc(n_rows):
    if n_rows not in _CACHE:
        _CACHE[n_rows] = _build(n_rows)
    return _CACHE[n_rows]


def _to_fp8(x):
    import ml_dtypes

    return np.ascontiguousarray(x, dtype=np.float32).astype(ml_dtypes.float8_e4m3)


def _to_bf16(x):
    import ml_dtypes

    return np.ascontiguousarray(x, dtype=np.float32).astype(ml_dtypes.bfloat16)


def _run(tgt, memory, Wq, Wk, Wv, trace=False):
    global LAST_RESULTS
    from concourse.bass_utils import run_bass_kernel_spmd

    n_rows = tgt.shape[1]
    nc = _get_nc(n_rows)

    tgt = np.ascontiguousarray(tgt, dtype=np.float32)
    memory = np.ascontiguousarray(memory, dtype=np.float32)
    tgt0t = _to_fp8(tgt[0].T)
    mem0t = _to_fp8(memory[0].T)

    in_maps = []
    for c in range(NCORES):
        sl = slice(c * CW, (c + 1) * CW)
        in_maps.append(
            {
                "tgt0t": tgt0t,
                "mem0t": mem0t,
                "wqt": _to_fp8(Wq[sl, :].T * WSCALE),
                "wkt": _to_fp8(Wk[sl, :].T * WSCALE),
                "wvt": _to_fp8(Wv[sl, :].T * WSCALE),
                "tgtc": _to_bf16(tgt[:, :, sl]),
            }
        )
    res = run_bass_kernel_spmd(nc, in_maps, list(range(NCORES)), trace=trace)
    LAST_RESULTS = res
    out = np.concatenate(
        [res.results[c]["outc"].astype(np.float32) for c in range(NCORES)], axis=2
    )
    return out


def kernel(tgt, memory, Wq, Wk, Wv):
    return _run(tgt, memory, Wq, Wk, Wv)
